# revision 1
# baseline (speedup 1.0000x reference)
"""Conv7x7(SAME) + LIF scan kernel for Trainium2, 8 NeuronCores.

Strategy:
- Shard H=512 spatially: core c owns output rows [64c, 64c+64). Host passes
  each core its 70-row input slab (64 + 3-row halo each side, zero padded),
  so no device-to-device communication is needed.
- Conv: 7x7 fp32 conv as 7 banded matmuls on the TensorEngine (band = the 7
  row-taps for one column-shift dx; column shifts realized as free-dim offsets
  into a width-padded SBUF tile). PSUM accumulates over dx. The two width
  halves run as col-tiled matmul pairs (tile_position) so M=64 doesn't waste
  the 128-wide PE array; output lands natively as [128, 256] = (half, row) x
  colchunk.
- LIF: bit-exact replication of the reference's per-op fp32 arithmetic on the
  VectorEngine: 6 ops per timestep on [128, 256] tiles.
    s = (i * 0.1) - i          (= -i_dec, exact negation)
    d = i - v
    v = (d * 0.1) + v          (= v_dec)
    z = (v - 1.0) > 0
    v = 0 where z              (copy_predicated reset)
    i = x_t - s                (= i_dec + x_t bitwise)
"""
import numpy as np
import concourse.bacc as bacc
import concourse.mybir as mybir
import concourse.tile as tile
from concourse.bass_utils import run_bass_kernel_spmd

T, H, WD, KK, PAD = 128, 512, 512, 7, 3
NCORES = 8
ROWS = H // NCORES            # 64 output rows per core
KP = ROWS + 2 * PAD           # 70 input rows per core
XB = 8                        # x tile buffers
ZB = 8                        # z staging buffers
NPS = 8                       # psum tiles in flight (PSUM = 8 banks)

_cached = None


def _build():
    global _cached
    if _cached is not None:
        return _cached

    f32 = mybir.dt.float32
    u32 = mybir.dt.uint32
    Alu = mybir.AluOpType

    nc = bacc.Bacc("TRN2", debug=False, num_devices=NCORES)
    xs_d = nc.dram_tensor("xs", (T, KP, WD), f32, kind="ExternalInput")
    bm_d = nc.dram_tensor("bm", (KP, KK * ROWS), f32, kind="ExternalInput")
    zs_d = nc.dram_tensor("zs", (T, ROWS, WD), f32, kind="ExternalOutput")

    with tile.TileContext(nc) as tc:
        with (
            tc.tile_pool(name="pool", bufs=1) as pool,
            tc.tile_pool(name="psum", bufs=1, space="PSUM") as psum,
        ):
            bm_t = pool.tile([KP, KK * ROWS], f32)
            nc.gpsimd.dma_start(bm_t[:], bm_d.ap())

            xts = [pool.tile([KP, WD + 2 * PAD], f32, name=f"xt{i}")
                   for i in range(XB)]
            for xt in xts:
                nc.gpsimd.memset(xt[:], 0.0)

            zts = [pool.tile([128, 256], f32, name=f"zt{i}") for i in range(ZB)]
            pss = [psum.tile([128, 256], f32, name=f"ps{i}") for i in range(NPS)]

            v_t = pool.tile([128, 256], f32)
            i_t = pool.tile([128, 256], f32)
            d_t = pool.tile([128, 256], f32)
            s_t = pool.tile([128, 256], f32)
            zero_t = pool.tile([128, 256], f32)
            nc.gpsimd.memset(v_t[:], 0.0)
            nc.gpsimd.memset(i_t[:], 0.0)
            nc.gpsimd.memset(zero_t[:], 0.0)

            for t in range(T):
                xt = xts[t % XB]
                nc.sync.dma_start(xt[:, PAD:PAD + WD], xs_d.ap()[t])
                ps = pss[t % NPS]
                for dx in range(KK):
                    for h in range(2):
                        nc.tensor.matmul(
                            ps[h * 64:(h + 1) * 64, :],
                            bm_t[:, dx * ROWS:(dx + 1) * ROWS],
                            xt[:, h * 256 + dx: h * 256 + dx + 256],
                            start=(dx == 0), stop=(dx == KK - 1),
                            tile_position=(0, h * 64),
                        )
                z_t = zts[t % ZB]
                # LIF step (all DVE, bit-exact vs reference order)
                nc.vector.scalar_tensor_tensor(
                    s_t[:], i_t[:], 0.1, i_t[:], Alu.mult, Alu.subtract)
                nc.vector.tensor_tensor(d_t[:], i_t[:], v_t[:], Alu.subtract)
                # psum read happens early so the bank frees for t+NPS
                nc.vector.tensor_tensor(i_t[:], ps[:], s_t[:], Alu.subtract)
                nc.vector.scalar_tensor_tensor(
                    v_t[:], d_t[:], 0.1, v_t[:], Alu.mult, Alu.add)
                nc.vector.tensor_scalar(
                    z_t[:], v_t[:], 1.0, 0.0, Alu.subtract, Alu.is_gt)
                nc.vector.copy_predicated(v_t[:], z_t[:].bitcast(u32), zero_t[:])

                nc.sync.dma_start(
                    zs_d.ap()[t].rearrange("r (h n) -> h r n", h=2), z_t[:])

    nc.compile()
    _cached = nc
    return nc


def _build_bmats(W):
    """bm[k, dx*64 + m] = W[dy=k-m, dx] for 0 <= k-m <= 6."""
    W = np.asarray(W, np.float32).reshape(KK, KK)
    bm = np.zeros((KP, KK * ROWS), np.float32)
    for dx in range(KK):
        for m in range(ROWS):
            for dy in range(KK):
                bm[m + dy, dx * ROWS + m] = W[dy, dx]
    return bm


def kernel(x, W):
    x = np.asarray(x, np.float32)
    nc = _build()
    bm = _build_bmats(W)
    xp = np.pad(x[:, 0], ((0, 0), (PAD, PAD), (0, 0)))  # [T, H+6, W]
    in_maps = []
    for c in range(NCORES):
        shard = np.ascontiguousarray(xp[:, c * ROWS: c * ROWS + KP, :])
        in_maps.append({"xs": shard, "bm": bm})
    res = run_bass_kernel_spmd(nc, in_maps, core_ids=list(range(NCORES)))
    z = np.concatenate([r["zs"] for r in res.results], axis=1)  # [T, H, W]
    return z.reshape(T, 1, H, WD).astype(np.float32)



# revision 2
# speedup vs baseline: 1.1528x; 1.1528x over previous
"""Conv7x7(SAME) + LIF scan kernel for Trainium2, 8 NeuronCores.

Strategy (v2):
- Shard H=512 spatially: core c owns output rows [64c, 64c+64). Host passes
  each core its 70-row input slab (64 + 3-row halo each side, zero padded),
  so no device-to-device communication is needed.
- Conv: 7x7 fp32 conv as 7 banded matmuls on the TensorEngine (band = the 7
  row-taps for one column-shift dx; column shifts realized as free-dim offsets
  into a width-padded SBUF tile). The two width halves run as col-tiled
  matmul pairs (tile_position) which issue concurrently on the PE array.
  Output lands as [128, 256] = (half, row) x colchunk.
- Timesteps are processed in chunks of 8 with dx as the outer loop inside a
  chunk, so the per-dx LDWEIGHTS cost amortizes over 8 timesteps instead of
  stalling every matmul group (the fp32 weight path cannot use FWL).
- The 0.1 (= DT*TAU_SYN_INV) factor is folded into the band matrix, so PSUM
  holds S_in = 0.1*conv(x) directly and the LIF needs only 4 DVE ops per
  timestep on the scaled synaptic state S = 0.1*i:
    v   = (v * 0.9) + S          (scalar_tensor_tensor, v_dec)
    S   = (S * 0.9) + psum       (scalar_tensor_tensor, reads PSUM)
    z   = (v - 1.0) > 0          (tensor_scalar -> bf16 tile, exact 0/1)
    v   = 0 where z              (copy_predicated reset)
- z is stored bf16 (exact for 0/1), halving output DMA; x-in and z-out ride
  different DMA queues (sync vs scalar) so they do not serialize.
"""
import numpy as np
import concourse.bacc as bacc
import concourse.mybir as mybir
import concourse.tile as tile
from concourse.bass_utils import run_bass_kernel_spmd

T, H, WD, KK, PAD = 128, 512, 512, 7, 3
NCORES = 8
ROWS = H // NCORES            # 64 output rows per core
KP = ROWS + 2 * PAD           # 70 input rows per core
TC = 8                        # timestep chunk (== PSUM banks in flight)
XB = 2 * TC                   # x tile buffers
ZB = 2 * TC                   # z staging buffers

_cached = None


def _build():
    global _cached
    if _cached is not None:
        return _cached

    f32 = mybir.dt.float32
    bf16 = mybir.dt.bfloat16
    u16 = mybir.dt.uint16
    Alu = mybir.AluOpType

    nc = bacc.Bacc("TRN2", debug=False, num_devices=NCORES)
    xs_d = nc.dram_tensor("xs", (T, KP, WD), f32, kind="ExternalInput")
    bm_d = nc.dram_tensor("bm", (KP, KK * ROWS), f32, kind="ExternalInput")
    zs_d = nc.dram_tensor("zs", (T, ROWS, WD), bf16, kind="ExternalOutput")

    with tile.TileContext(nc) as tc:
        with (
            tc.tile_pool(name="pool", bufs=1) as pool,
            tc.tile_pool(name="psum", bufs=1, space="PSUM") as psum,
        ):
            bm_t = pool.tile([KP, KK * ROWS], f32)
            nc.gpsimd.dma_start(bm_t[:], bm_d.ap())

            xts = [pool.tile([KP, WD + 2 * PAD], f32, name=f"xt{i}")
                   for i in range(XB)]
            for xt in xts:
                nc.gpsimd.memset(xt[:], 0.0)

            zts = [pool.tile([128, 256], bf16, name=f"zt{i}") for i in range(ZB)]
            pss = [psum.tile([128, 256], f32, name=f"ps{i}") for i in range(TC)]

            v_t = pool.tile([128, 256], f32)
            s_t = pool.tile([128, 256], f32)
            zero_t = pool.tile([128, 256], f32)
            nc.gpsimd.memset(v_t[:], 0.0)
            nc.gpsimd.memset(s_t[:], 0.0)
            nc.gpsimd.memset(zero_t[:], 0.0)

            for t0 in range(0, T, TC):
                # prefetch the chunk's x slabs
                for t in range(t0, t0 + TC):
                    xt = xts[t % XB]
                    nc.sync.dma_start(xt[:, PAD:PAD + WD], xs_d.ap()[t])
                # conv: dx outer so LDWEIGHTS for a band loads once per chunk
                for dx in range(KK):
                    for t in range(t0, t0 + TC):
                        xt = xts[t % XB]
                        ps = pss[t % TC]
                        for h in range(2):
                            nc.tensor.matmul(
                                ps[h * 64:(h + 1) * 64, :],
                                bm_t[:, dx * ROWS:(dx + 1) * ROWS],
                                xt[:, h * 256 + dx: h * 256 + dx + 256],
                                start=(dx == 0), stop=(dx == KK - 1),
                                tile_position=(0, h * 64),
                            )
                # LIF steps for the chunk
                for t in range(t0, t0 + TC):
                    ps = pss[t % TC]
                    z_t = zts[t % ZB]
                    nc.vector.scalar_tensor_tensor(
                        v_t[:], v_t[:], 0.9, s_t[:], Alu.mult, Alu.add)
                    nc.vector.scalar_tensor_tensor(
                        s_t[:], s_t[:], 0.9, ps[:], Alu.mult, Alu.add)
                    nc.vector.tensor_scalar(
                        z_t[:], v_t[:], 1.0, 0.0, Alu.subtract, Alu.is_gt)
                    nc.vector.copy_predicated(
                        v_t[:], z_t[:].bitcast(u16), zero_t[:])
                    nc.scalar.dma_start(
                        zs_d.ap()[t].rearrange("r (h n) -> h r n", h=2), z_t[:])

    nc.compile()
    _cached = nc
    return nc


def _build_bmats(W):
    """bm[k, dx*64 + m] = 0.1 * W[dy=k-m, dx] for 0 <= k-m <= 6."""
    W = np.asarray(W, np.float32).reshape(KK, KK)
    bm = np.zeros((KP, KK * ROWS), np.float32)
    for dx in range(KK):
        for m in range(ROWS):
            for dy in range(KK):
                bm[m + dy, dx * ROWS + m] = np.float32(0.1) * W[dy, dx]
    return bm


def kernel(x, W):
    x = np.asarray(x, np.float32)
    nc = _build()
    bm = _build_bmats(W)
    xp = np.pad(x[:, 0], ((0, 0), (PAD, PAD), (0, 0)))  # [T, H+6, W]
    in_maps = []
    for c in range(NCORES):
        shard = np.ascontiguousarray(xp[:, c * ROWS: c * ROWS + KP, :])
        in_maps.append({"xs": shard, "bm": bm})
    res = run_bass_kernel_spmd(nc, in_maps, core_ids=list(range(NCORES)))
    z = np.concatenate([r["zs"].astype(np.float32) for r in res.results],
                       axis=1)  # [T, H, W]
    return z.reshape(T, 1, H, WD).astype(np.float32)


# revision 5
# speedup vs baseline: 1.9112x; 1.6579x over previous
"""Conv7x7(SAME) + LIF scan kernel for Trainium2, 8 NeuronCores.

Strategy (v3):
- Shard H=512 spatially: core c owns output rows [64c, 64c+64). Host passes
  each core its 70-row input slab (64 + 3-row halo each side, zero padded).
- Column-parity packing: output cols w = 2c+psi. Pairing columns lets one
  banded matmul cover TWO dx taps (the parity bit selects which of the
  duplicated row-sets contributes), so the 7 dx shifts collapse into 4
  pair-offset matmul groups. Input is stored column-deinterleaved
  (partition = (phi, row)), prepared host-side so the DMA loads it directly.
- Output rows are processed as two 32-row chunks; (chunk, psi) gives 4
  independent M=32 matmuls per offset group, packed into the 128-wide PE
  array via 4-way col-tiling (tile_position) — they issue concurrently.
- Precision: fp16 3-pass hi/lo split (exact to ~2e-7 relative):
    P1: Bh  (x) xh          P2: Bh*2^-6 (x) xl*2^6       P3: Bl (x) xh
  where B = 51.2*W (the 0.1 LIF current scale and a 512x state rescale are
  folded in so all fp16 values are normal; threshold becomes 512). Bh/Bl is
  an exact two-term fp16 split with Bl forced normal by directed rounding.
  fp16 matmuls run 1 cycle/row and their weight loads hide (unlike fp32,
  which reloads weights every matmul on the slow path: measured +390ns/MM).
- LIF per timestep, 4 ops on [128,256] tiles (V = 512*v, S = 512*0.1*i):
    V = (V * 0.9) + S; S = (S * 0.9) + psum; z = (V-512)>0 (bf16); V=0 where z
- z is stored bf16 in the packed parity layout; the host un-permutes.
"""
import numpy as np
import concourse.bacc as bacc
import concourse.mybir as mybir
import concourse.tile as tile
from concourse.bass_utils import run_bass_kernel_spmd

T, H, WD, KK, PAD = 128, 512, 512, 7, 3
NCORES = 8
ROWS = H // NCORES            # 64 output rows per core
KP = ROWS + 2 * PAD           # 70 input rows per core
CH = 32                       # row chunk
KQ = 2 * (CH + 6)             # 76 partitions per parity tile
NP = (WD + 2 * PAD + 1) // 2  # 259 column pairs
NPP = 260                     # padded pair columns in SBUF
NC2 = WD // 2                 # 256 output column pairs
TC = 8                        # timestep chunk (PSUM tiles in flight)
XB = 2 * TC
ZB = 2 * TC
SCL = np.float32(512.0)

_cached = None


def _build():
    global _cached
    if _cached is not None:
        return _cached

    f32 = mybir.dt.float32
    f16 = mybir.dt.float16
    bf16 = mybir.dt.bfloat16
    u16 = mybir.dt.uint16
    Alu = mybir.AluOpType

    nc = bacc.Bacc("TRN2", debug=False, num_devices=NCORES)
    # host-prepared parity tensors: [T, 2(phi), KP, NPP] fp16
    xh_d = nc.dram_tensor("xh", (T, 2, KP, NPP), f16, kind="ExternalInput")
    xl_d = nc.dram_tensor("xl", (T, 2, KP, NPP), f16, kind="ExternalInput")
    # three band matrices, each [KQ, 8*CH] (col block = (psi*4+o)*32 + m)
    bmh_d = nc.dram_tensor("bmh", (KQ, 8 * CH), f16, kind="ExternalInput")
    bm2_d = nc.dram_tensor("bm2", (KQ, 8 * CH), f16, kind="ExternalInput")
    bml_d = nc.dram_tensor("bml", (KQ, 8 * CH), f16, kind="ExternalInput")
    zs_d = nc.dram_tensor("zs", (T, 128, NC2), bf16, kind="ExternalOutput")

    with tile.TileContext(nc) as tc:
        with (
            tc.tile_pool(name="pool", bufs=1) as pool,
            tc.tile_pool(name="psum", bufs=1, space="PSUM") as psum,
        ):
            bms = []
            for name, d in (("bmh", bmh_d), ("bm2", bm2_d), ("bml", bml_d)):
                t_ = pool.tile([KQ, 8 * CH], f16, name=name + "_t")
                nc.gpsimd.dma_start(t_[:], d.ap())
                bms.append(t_)

            # x parity tiles: per timestep 2 chunks x {hi, lo}
            xts = [[[pool.tile([KQ, NPP], f16, name=f"x{hl}{k}_{i}")
                     for k in range(2)] for hl in range(2)]
                   for i in range(XB)]

            zts = [pool.tile([128, NC2], bf16, name=f"zt{i}") for i in range(ZB)]
            pss = [psum.tile([128, NC2], f32, name=f"ps{i}") for i in range(TC)]

            v_t = pool.tile([128, NC2], f32)
            s_t = pool.tile([128, NC2], f32)
            zero_t = pool.tile([128, NC2], f32)
            nc.gpsimd.memset(v_t[:], 0.0)
            nc.gpsimd.memset(s_t[:], 0.0)
            nc.gpsimd.memset(zero_t[:], 0.0)

            def load_x(t):
                bufs = xts[t % XB]
                for k in range(2):
                    # partitions (phi*38 + rr) <- host[phi, 32k+rr, :]
                    r0 = 32 * k
                    for phi in range(2):
                        nc.sync.dma_start(
                            bufs[0][k][phi * (CH + 6):(phi + 1) * (CH + 6), :],
                            xh_d.ap()[t, phi, r0:r0 + CH + 6, :])
                        nc.gpsimd.dma_start(
                            bufs[1][k][phi * (CH + 6):(phi + 1) * (CH + 6), :],
                            xl_d.ap()[t, phi, r0:r0 + CH + 6, :])

            for t0 in range(0, T, TC):
                for t in range(t0, t0 + TC):
                    load_x(t)
                # conv: offset-group outer, then pass, then timestep
                for o in range(4):
                    for p in range(3):
                        bm = bms[p]
                        hl = 1 if p == 1 else 0
                        for t in range(t0, t0 + TC):
                            bufs = xts[t % XB]
                            ps = pss[t % TC]
                            for k in range(2):
                                for psi in range(2):
                                    cg = 32 * (2 * k + psi)
                                    nc.tensor.matmul(
                                        ps[cg:cg + 32, :],
                                        bm[:, (psi * 4 + o) * CH:
                                           (psi * 4 + o + 1) * CH],
                                        bufs[hl][k][:, o:o + NC2],
                                        start=(o == 0 and p == 0),
                                        stop=(o == 3 and p == 2),
                                        tile_position=(0, cg),
                                    )
                # LIF steps
                for t in range(t0, t0 + TC):
                    ps = pss[t % TC]
                    z_t = zts[t % ZB]
                    nc.vector.scalar_tensor_tensor(
                        v_t[:], v_t[:], 0.9, s_t[:], Alu.mult, Alu.add)
                    nc.vector.scalar_tensor_tensor(
                        s_t[:], s_t[:], 0.9, ps[:], Alu.mult, Alu.add)
                    nc.vector.tensor_scalar(
                        z_t[:], v_t[:], 512.0, 0.0, Alu.subtract, Alu.is_gt)
                    nc.vector.copy_predicated(
                        v_t[:], z_t[:].bitcast(u16), zero_t[:])
                    nc.scalar.dma_start(zs_d.ap()[t], z_t[:])

    nc.compile()
    _cached = nc
    return nc


def _split_fp16_normal(a):
    """Exact two-term fp16 split a = hi + lo with lo normal-or-zero."""
    hi = a.astype(np.float16)
    lo = a - hi.astype(np.float32)
    # force |lo| >= fp16 min normal (or 0) by moving hi one ulp
    tiny = (np.abs(lo) < 6.2e-5) & (lo != 0.0)
    hi2 = np.nextafter(hi, np.where(lo > 0, np.float16(-np.inf),
                                    np.float16(np.inf)).astype(np.float16))
    hi = np.where(tiny, hi2, hi)
    lo = (a - hi.astype(np.float32)).astype(np.float16)
    return hi, lo


def _build_bmats(W):
    """Parity band matrices B[psi][o]: [KQ, 32] with two dx taps folded."""
    W = np.asarray(W, np.float32).reshape(KK, KK)
    B = np.zeros((KQ, 8 * CH), np.float32)
    for psi in range(2):
        for dx in range(KK):
            o = (psi + dx) >> 1
            phi = (psi + dx) & 1
            col0 = (psi * 4 + o) * CH
            for dy in range(KK):
                for m in range(CH):
                    B[phi * (CH + 6) + m + dy, col0 + m] += \
                        np.float32(0.1) * SCL * W[dy, dx]
    bh, bl = _split_fp16_normal(B)
    b2 = (bh.astype(np.float32) * np.float32(2.0 ** -6)).astype(np.float16)
    return bh, b2, bl


def _prep_x(x):
    """Pad, parity-split and hi/lo-split x -> xh, xl [T, 2, H+6, NPP] fp16."""
    xp = np.pad(x[:, 0], ((0, 0), (PAD, PAD), (PAD, 2 * NPP - WD - PAD)))
    xh = xp.astype(np.float16)
    xl = ((xp - xh.astype(np.float32)) * np.float32(2.0 ** 6)).astype(
        np.float16)
    # [T, rows, 2*NPP] -> [T, 2, rows, NPP]
    def par(a):
        return np.ascontiguousarray(
            a.reshape(a.shape[0], a.shape[1], NPP, 2).transpose(0, 3, 1, 2))
    return par(xh), par(xl)


def kernel(x, W):
    x = np.asarray(x, np.float32)
    nc = _build()
    bh, b2, bl = _build_bmats(W)
    xh, xl = _prep_x(x)  # [T, 2, H+6, NPP]
    in_maps = []
    for c in range(NCORES):
        sh = np.ascontiguousarray(xh[:, :, c * ROWS: c * ROWS + KP, :])
        sl = np.ascontiguousarray(xl[:, :, c * ROWS: c * ROWS + KP, :])
        in_maps.append({"xh": sh, "xl": sl, "bmh": bh, "bm2": b2, "bml": bl})
    res = run_bass_kernel_spmd(nc, in_maps, core_ids=list(range(NCORES)))
    # un-permute: z[t, 32*(2k+psi)+m, c] -> out[t, 32k+m, 2c+psi]
    outs = []
    for r in res.results:
        z = r["zs"].astype(np.float32).reshape(T, 2, 2, CH, NC2)
        outs.append(z.transpose(0, 1, 3, 4, 2).reshape(T, ROWS, WD))
    z = np.concatenate(outs, axis=1)  # [T, H, W]
    return z.reshape(T, 1, H, WD).astype(np.float32)


# revision 9
# speedup vs baseline: 2.6091x; 1.3651x over previous
"""Conv7x7(SAME) + LIF scan kernel for Trainium2, 8 NeuronCores.

Strategy (v3):
- Shard H=512 spatially: core c owns output rows [64c, 64c+64). Host passes
  each core its 70-row input slab (64 + 3-row halo each side, zero padded).
- Column-parity packing: output cols w = 2c+psi. Pairing columns lets one
  banded matmul cover TWO dx taps (the parity bit selects which of the
  duplicated row-sets contributes), so the 7 dx shifts collapse into 4
  pair-offset matmul groups. Input is stored column-deinterleaved
  (partition = (phi, row)), prepared host-side so the DMA loads it directly.
- Output rows are processed as two 32-row chunks; (chunk, psi) gives 4
  independent M=32 matmuls per offset group, packed into the 128-wide PE
  array via 4-way col-tiling (tile_position) — they issue concurrently.
- Precision: fp16 3-pass hi/lo split (exact to ~2e-7 relative):
    P1: Bh  (x) xh          P2: Bh*2^-6 (x) xl*2^6       P3: Bl (x) xh
  where B = 51.2*W (the 0.1 LIF current scale and a 512x state rescale are
  folded in so all fp16 values are normal; threshold becomes 512). Bh/Bl is
  an exact two-term fp16 split with Bl forced normal by directed rounding.
  fp16 matmuls run 1 cycle/row and their weight loads hide (unlike fp32,
  which reloads weights every matmul on the slow path: measured +390ns/MM).
- LIF per timestep, 4 ops on [128,256] tiles (V = 512*v, S = 512*0.1*i):
    V = (V * 0.9) + S; S = (S * 0.9) + psum; z = (V-512)>0 (bf16); V=0 where z
- z is stored bf16 in the packed parity layout; the host un-permutes.
"""
import numpy as np
import concourse.bacc as bacc
import concourse.mybir as mybir
import concourse.tile as tile
from concourse.bass_utils import run_bass_kernel_spmd

T, H, WD, KK, PAD = 128, 512, 512, 7, 3
NCORES = 8
ROWS = H // NCORES            # 64 output rows per core
KP = ROWS + 2 * PAD           # 70 input rows per core
CH = 32                       # row chunk
KQ = 2 * (CH + 6)             # 76 partitions per parity tile
NP = (WD + 2 * PAD + 1) // 2  # 259 column pairs
NPP = 260                     # padded pair columns in SBUF
NC2 = WD // 2                 # 256 output column pairs
TC = 8                        # timestep chunk (PSUM tiles in flight)
XB = 2 * TC
ZB = 2 * TC
SCL = np.float32(512.0)

_cached = None


def _build():
    global _cached
    if _cached is not None:
        return _cached

    f32 = mybir.dt.float32
    f16 = mybir.dt.float16
    bf16 = mybir.dt.bfloat16
    u16 = mybir.dt.uint16
    Alu = mybir.AluOpType

    nc = bacc.Bacc("TRN2", debug=False, num_devices=NCORES)
    # host-prepared parity tensors: [T, 2(chunk), 2(phi), 38, NPP] fp16
    xh_d = nc.dram_tensor("xh", (T, 2, 2, CH + 6, NPP), f16,
                          kind="ExternalInput")
    xl_d = nc.dram_tensor("xl", (T, 2, 2, CH + 6, NPP), f16,
                          kind="ExternalInput")
    # three band matrices, each [KQ, 8*CH] (col block = (psi*4+o)*32 + m)
    bmh_d = nc.dram_tensor("bmh", (KQ, 8 * CH), f16, kind="ExternalInput")
    bm2_d = nc.dram_tensor("bm2", (KQ, 8 * CH), f16, kind="ExternalInput")
    bml_d = nc.dram_tensor("bml", (KQ, 8 * CH), f16, kind="ExternalInput")
    zs_d = nc.dram_tensor("zs", (T, 128, NC2), bf16, kind="ExternalOutput")

    with tile.TileContext(nc) as tc:
        with (
            tc.tile_pool(name="pool", bufs=1) as pool,
            tc.tile_pool(name="psum", bufs=1, space="PSUM") as psum,
        ):
            bms = []
            for name, d in (("bmh", bmh_d), ("bm2", bm2_d), ("bml", bml_d)):
                t_ = pool.tile([KQ, 8 * CH], f16, name=name + "_t")
                nc.gpsimd.dma_start(t_[:], d.ap())
                bms.append(t_)

            # x parity tiles: per timestep 2 chunks x {hi, lo}
            xts = [[[pool.tile([KQ, NPP], f16, name=f"x{hl}{k}_{i}")
                     for k in range(2)] for hl in range(2)]
                   for i in range(XB)]

            zts = [pool.tile([128, NC2], bf16, name=f"zt{i}") for i in range(ZB)]
            pss = [psum.tile([128, NC2], f32, name=f"ps{i}") for i in range(TC)]

            v_t = pool.tile([128, NC2], f32)
            s_t = pool.tile([128, NC2], f32)
            zero_t = pool.tile([128, NC2], f32)
            nc.gpsimd.memset(v_t[:], 0.0)
            nc.gpsimd.memset(s_t[:], 0.0)
            nc.gpsimd.memset(zero_t[:], 0.0)

            for t in range(T):
                bufs = xts[t % XB]
                for k in range(2):
                    # partitions (phi*38 + rr) <- host[k, phi, rr, :]
                    nc.sync.dma_start(
                        bufs[0][k][:],
                        xh_d.ap()[t, k].rearrange("p r n -> (p r) n"))
                    nc.gpsimd.dma_start(
                        bufs[1][k][:],
                        xl_d.ap()[t, k].rearrange("p r n -> (p r) n"))
                ps = pss[t % TC]
                for o in range(4):
                    for p in range(3):
                        bm = bms[p]
                        hl = 1 if p == 1 else 0
                        for k in range(2):
                            for psi in range(2):
                                cg = 32 * (2 * k + psi)
                                nc.tensor.matmul(
                                    ps[cg:cg + 32, :],
                                    bm[:, (psi * 4 + o) * CH:
                                       (psi * 4 + o + 1) * CH],
                                    bufs[hl][k][:, o:o + NC2],
                                    start=(o == 0 and p == 0),
                                    stop=(o == 3 and p == 2),
                                    tile_position=(0, cg),
                                )
                z_t = zts[t % ZB]
                nc.vector.scalar_tensor_tensor(
                    v_t[:], v_t[:], 0.9, s_t[:], Alu.mult, Alu.add)
                nc.vector.scalar_tensor_tensor(
                    s_t[:], s_t[:], 0.9, ps[:], Alu.mult, Alu.add)
                nc.vector.tensor_scalar(
                    z_t[:], v_t[:], 512.0, 0.0, Alu.subtract, Alu.is_gt)
                nc.vector.copy_predicated(
                    v_t[:], z_t[:].bitcast(u16), zero_t[:])
                nc.scalar.dma_start(zs_d.ap()[t], z_t[:])

    nc.compile()
    _cached = nc
    return nc


def _split_fp16_normal(a):
    """Exact two-term fp16 split a = hi + lo with lo normal-or-zero."""
    hi = a.astype(np.float16)
    lo = a - hi.astype(np.float32)
    # force |lo| >= fp16 min normal (or 0) by moving hi one ulp
    tiny = (np.abs(lo) < 6.2e-5) & (lo != 0.0)
    hi2 = np.nextafter(hi, np.where(lo > 0, np.float16(-np.inf),
                                    np.float16(np.inf)).astype(np.float16))
    hi = np.where(tiny, hi2, hi)
    lo = (a - hi.astype(np.float32)).astype(np.float16)
    return hi, lo


def _build_bmats(W):
    """Parity band matrices B[psi][o]: [KQ, 32] with two dx taps folded."""
    W = np.asarray(W, np.float32).reshape(KK, KK)
    B = np.zeros((KQ, 8 * CH), np.float32)
    for psi in range(2):
        for dx in range(KK):
            o = (psi + dx) >> 1
            phi = (psi + dx) & 1
            col0 = (psi * 4 + o) * CH
            for dy in range(KK):
                for m in range(CH):
                    B[phi * (CH + 6) + m + dy, col0 + m] += \
                        np.float32(0.1) * SCL * W[dy, dx]
    bh, bl = _split_fp16_normal(B)
    b2 = (bh.astype(np.float32) * np.float32(2.0 ** -6)).astype(np.float16)
    return bh, b2, bl


def _prep_x(x):
    """Pad, parity-split and hi/lo-split x -> xh, xl [T, 2, H+6, NPP] fp16."""
    xp = np.pad(x[:, 0], ((0, 0), (PAD, PAD), (PAD, 2 * NPP - WD - PAD)))
    xh = xp.astype(np.float16)
    xl = ((xp - xh.astype(np.float32)) * np.float32(2.0 ** 6)).astype(
        np.float16)
    # [T, rows, 2*NPP] -> [T, 2(phi), rows, NPP]
    def par(a):
        return np.ascontiguousarray(
            a.reshape(a.shape[0], a.shape[1], NPP, 2).transpose(0, 3, 1, 2))
    return par(xh), par(xl)


def _chunked(a, c):
    """[T, 2(phi), KP, NPP] core slab -> [T, 2(k), 2(phi), 38, NPP]."""
    s = a[:, :, c * ROWS: c * ROWS + KP, :]
    return np.ascontiguousarray(
        np.stack([s[:, :, 0:CH + 6, :], s[:, :, CH:CH + CH + 6, :]], axis=1))


def kernel(x, W):
    x = np.asarray(x, np.float32)
    nc = _build()
    bh, b2, bl = _build_bmats(W)
    xh, xl = _prep_x(x)  # [T, 2, H+6, NPP]
    in_maps = []
    for c in range(NCORES):
        in_maps.append({"xh": _chunked(xh, c), "xl": _chunked(xl, c),
                        "bmh": bh, "bm2": b2, "bml": bl})
    res = run_bass_kernel_spmd(nc, in_maps, core_ids=list(range(NCORES)))
    # un-permute: z[t, 32*(2k+psi)+m, c] -> out[t, 32k+m, 2c+psi]
    outs = []
    for r in res.results:
        z = r["zs"].astype(np.float32).reshape(T, 2, 2, CH, NC2)
        outs.append(z.transpose(0, 1, 3, 4, 2).reshape(T, ROWS, WD))
    z = np.concatenate(outs, axis=1)  # [T, H, W]
    return z.reshape(T, 1, H, WD).astype(np.float32)


# revision 11
# speedup vs baseline: 2.6269x; 1.0068x over previous
"""Conv7x7(SAME) + LIF scan kernel for Trainium2, 8 NeuronCores.

Strategy (v3):
- Shard H=512 spatially: core c owns output rows [64c, 64c+64). Host passes
  each core its 70-row input slab (64 + 3-row halo each side, zero padded).
- Column-parity packing: output cols w = 2c+psi. Pairing columns lets one
  banded matmul cover TWO dx taps (the parity bit selects which of the
  duplicated row-sets contributes), so the 7 dx shifts collapse into 4
  pair-offset matmul groups. Input is stored column-deinterleaved
  (partition = (phi, row)), prepared host-side so the DMA loads it directly.
- Output rows are processed as two 32-row chunks; (chunk, psi) gives 4
  independent M=32 matmuls per offset group, packed into the 128-wide PE
  array via 4-way col-tiling (tile_position) — they issue concurrently.
- Precision: fp16 3-pass hi/lo split (exact to ~2e-7 relative):
    P1: Bh  (x) xh          P2: Bh*2^-6 (x) xl*2^6       P3: Bl (x) xh
  where B = 51.2*W (the 0.1 LIF current scale and a 512x state rescale are
  folded in so all fp16 values are normal; threshold becomes 512). Bh/Bl is
  an exact two-term fp16 split with Bl forced normal by directed rounding.
  fp16 matmuls run 1 cycle/row and their weight loads hide (unlike fp32,
  which reloads weights every matmul on the slow path: measured +390ns/MM).
- LIF per timestep, 4 ops on [128,256] tiles (V = 512*v, S = 512*0.1*i):
    V = (V * 0.9) + S; S = (S * 0.9) + psum; z = (V-512)>0 (bf16); V=0 where z
- z is stored bf16 in the packed parity layout; the host un-permutes.
"""
import numpy as np
import concourse.bacc as bacc
import concourse.mybir as mybir
import concourse.tile as tile
from concourse.bass_utils import run_bass_kernel_spmd

T, H, WD, KK, PAD = 128, 512, 512, 7, 3
NCORES = 8
ROWS = H // NCORES            # 64 output rows per core
KP = ROWS + 2 * PAD           # 70 input rows per core
CH = 32                       # row chunk
KQ = 2 * (CH + 6)             # 76 partitions per parity tile
NP = (WD + 2 * PAD + 1) // 2  # 259 column pairs
NPP = 260                     # padded pair columns in SBUF
NC2 = WD // 2                 # 256 output column pairs
TC = 8                        # timestep chunk (PSUM tiles in flight)
XB = 2 * TC
ZB = 2 * TC
SCL = np.float32(512.0)

_cached = None


def _build():
    global _cached
    if _cached is not None:
        return _cached

    f32 = mybir.dt.float32
    f16 = mybir.dt.float16
    bf16 = mybir.dt.bfloat16
    u16 = mybir.dt.uint16
    Alu = mybir.AluOpType

    nc = bacc.Bacc("TRN2", debug=False, num_devices=NCORES)
    # host-prepared parity tensors: [T, 2(chunk), 2(phi), 38, NPP] fp16
    xh_d = nc.dram_tensor("xh", (T, 2, 2, CH + 6, NPP), f16,
                          kind="ExternalInput")
    xl_d = nc.dram_tensor("xl", (T, 2, 2, CH + 6, NPP), f16,
                          kind="ExternalInput")
    # three band matrices, each [KQ, 8*CH] (col block = (psi*4+o)*32 + m)
    bmh_d = nc.dram_tensor("bmh", (KQ, 8 * CH), f16, kind="ExternalInput")
    bm2_d = nc.dram_tensor("bm2", (KQ, 8 * CH), f16, kind="ExternalInput")
    bml_d = nc.dram_tensor("bml", (KQ, 8 * CH), f16, kind="ExternalInput")
    zs_d = nc.dram_tensor("zs", (T, 128, NC2), bf16, kind="ExternalOutput")

    with tile.TileContext(nc) as tc:
        with (
            tc.tile_pool(name="pool", bufs=1) as pool,
            tc.tile_pool(name="psum", bufs=1, space="PSUM") as psum,
        ):
            bms = []
            for name, d in (("bmh", bmh_d), ("bm2", bm2_d), ("bml", bml_d)):
                t_ = pool.tile([KQ, 8 * CH], f16, name=name + "_t")
                nc.scalar.dma_start(t_[:], d.ap())
                bms.append(t_)

            # x parity tiles: per timestep 2 chunks x {hi, lo}
            xts = [[[pool.tile([KQ, NPP], f16, name=f"x{hl}{k}_{i}")
                     for k in range(2)] for hl in range(2)]
                   for i in range(XB)]

            zts = [pool.tile([128, NC2], bf16, name=f"zt{i}") for i in range(ZB)]
            pss = [psum.tile([128, NC2], f32, name=f"ps{i}") for i in range(TC)]

            v_t = pool.tile([128, NC2], f32)
            s_t = pool.tile([128, NC2], f32)
            zero_t = pool.tile([128, NC2], f32)
            nc.vector.memset(v_t[:], 0.0)
            nc.vector.memset(s_t[:], 0.0)
            nc.vector.memset(zero_t[:], 0.0)

            for t in range(T):
                bufs = xts[t % XB]
                for k in range(2):
                    # partitions (phi*38 + rr) <- host[k, phi, rr, :]
                    nc.sync.dma_start(
                        bufs[0][k][:],
                        xh_d.ap()[t, k].rearrange("p r n -> (p r) n"))
                    nc.gpsimd.dma_start(
                        bufs[1][k][:],
                        xl_d.ap()[t, k].rearrange("p r n -> (p r) n"))
                ps = pss[t % TC]
                for o in range(4):
                    for p in range(3):
                        bm = bms[p]
                        hl = 1 if p == 1 else 0
                        for k in range(2):
                            for psi in range(2):
                                cg = 32 * (2 * k + psi)
                                nc.tensor.matmul(
                                    ps[cg:cg + 32, :],
                                    bm[:, (psi * 4 + o) * CH:
                                       (psi * 4 + o + 1) * CH],
                                    bufs[hl][k][:, o:o + NC2],
                                    start=(o == 0 and p == 0),
                                    stop=(o == 3 and p == 2),
                                    tile_position=(0, cg),
                                )
                z_t = zts[t % ZB]
                nc.vector.scalar_tensor_tensor(
                    v_t[:], v_t[:], 0.9, s_t[:], Alu.mult, Alu.add)
                nc.vector.scalar_tensor_tensor(
                    s_t[:], s_t[:], 0.9, ps[:], Alu.mult, Alu.add)
                nc.vector.tensor_scalar(
                    z_t[:], v_t[:], 512.0, 0.0, Alu.subtract, Alu.is_gt)
                nc.vector.copy_predicated(
                    v_t[:], z_t[:].bitcast(u16), zero_t[:])
                nc.scalar.dma_start(zs_d.ap()[t], z_t[:])

    nc.compile()
    _cached = nc
    return nc


def _split_fp16_normal(a):
    """Exact two-term fp16 split a = hi + lo with lo normal-or-zero."""
    hi = a.astype(np.float16)
    lo = a - hi.astype(np.float32)
    # force |lo| >= fp16 min normal (or 0) by moving hi one ulp
    tiny = (np.abs(lo) < 6.2e-5) & (lo != 0.0)
    hi2 = np.nextafter(hi, np.where(lo > 0, np.float16(-np.inf),
                                    np.float16(np.inf)).astype(np.float16))
    hi = np.where(tiny, hi2, hi)
    lo = (a - hi.astype(np.float32)).astype(np.float16)
    return hi, lo


def _build_bmats(W):
    """Parity band matrices B[psi][o]: [KQ, 32] with two dx taps folded."""
    W = np.asarray(W, np.float32).reshape(KK, KK)
    B = np.zeros((KQ, 8 * CH), np.float32)
    for psi in range(2):
        for dx in range(KK):
            o = (psi + dx) >> 1
            phi = (psi + dx) & 1
            col0 = (psi * 4 + o) * CH
            for dy in range(KK):
                for m in range(CH):
                    B[phi * (CH + 6) + m + dy, col0 + m] += \
                        np.float32(0.1) * SCL * W[dy, dx]
    bh, bl = _split_fp16_normal(B)
    b2 = (bh.astype(np.float32) * np.float32(2.0 ** -6)).astype(np.float16)
    return bh, b2, bl


def _prep_x(x):
    """Pad, parity-split and hi/lo-split x -> xh, xl [T, 2, H+6, NPP] fp16."""
    xp = np.pad(x[:, 0], ((0, 0), (PAD, PAD), (PAD, 2 * NPP - WD - PAD)))
    xh = xp.astype(np.float16)
    xl = ((xp - xh.astype(np.float32)) * np.float32(2.0 ** 6)).astype(
        np.float16)
    # [T, rows, 2*NPP] -> [T, 2(phi), rows, NPP]
    def par(a):
        return np.ascontiguousarray(
            a.reshape(a.shape[0], a.shape[1], NPP, 2).transpose(0, 3, 1, 2))
    return par(xh), par(xl)


def _chunked(a, c):
    """[T, 2(phi), KP, NPP] core slab -> [T, 2(k), 2(phi), 38, NPP]."""
    s = a[:, :, c * ROWS: c * ROWS + KP, :]
    return np.ascontiguousarray(
        np.stack([s[:, :, 0:CH + 6, :], s[:, :, CH:CH + CH + 6, :]], axis=1))


def kernel(x, W):
    x = np.asarray(x, np.float32)
    nc = _build()
    bh, b2, bl = _build_bmats(W)
    xh, xl = _prep_x(x)  # [T, 2, H+6, NPP]
    in_maps = []
    for c in range(NCORES):
        in_maps.append({"xh": _chunked(xh, c), "xl": _chunked(xl, c),
                        "bmh": bh, "bm2": b2, "bml": bl})
    res = run_bass_kernel_spmd(nc, in_maps, core_ids=list(range(NCORES)))
    # un-permute: z[t, 32*(2k+psi)+m, c] -> out[t, 32k+m, 2c+psi]
    outs = []
    for r in res.results:
        z = r["zs"].astype(np.float32).reshape(T, 2, 2, CH, NC2)
        outs.append(z.transpose(0, 1, 3, 4, 2).reshape(T, ROWS, WD))
    z = np.concatenate(outs, axis=1)  # [T, H, W]
    return z.reshape(T, 1, H, WD).astype(np.float32)
